# revision 23
# baseline (speedup 1.0000x reference)
"""Multi-head self-attention (B=2, N=2048, C=1024, H=16) on 8 trn2 NeuronCores.

Sharding: core i computes heads {2i, 2i+1} for both batches (head-parallel
attention), then one 8-way AllToAll redistributes attention outputs so core i
holds the full channel dim for output rows (b = i//4, seq chunk 512*(i%4)).
Each core then computes its own 512-row slice of the output projection.
Host work is layout only: dtype cast, transpose, slice, concatenate.
"""

import numpy as np
import ml_dtypes
import bass_rust

import concourse.bass as bass
import concourse.mybir as mybir
import concourse.tile as tile
from concourse.bass_utils import run_bass_kernel_spmd

B, N, C = 2, 2048, 1024
H = 16
D = C // H           # 64
W = 8                # cores
HL = 2               # heads per core
P = 128
KT = C // P          # 8 k-tiles over channels
NQC = N // 512       # 4 query chunks of 512 per batch
SCALE = float(D) ** -0.5

F32 = mybir.dt.float32
BF16 = mybir.dt.bfloat16
BF = ml_dtypes.bfloat16


_RING_INSTS = (
    mybir.InstDMACopy, mybir.InstDMA, mybir.InstTensorLoad, mybir.InstTensorSave,
    mybir.InstDmaTransposeAnt, mybir.InstDMAGatherAnt, mybir.InstDMAScatterAddAnt,
    mybir.InstCollectiveCompute,
)


def _split_multiwait(nc: bass.Bass, gate_sems: dict) -> None:
    """This toolchain's walrus codegen accepts at most ONE sync wait per
    instruction, but the Tile scheduler attaches several.

    Compute/CTRL instructions: move all but the last wait onto EventSemaphore
    instructions inserted just before them on the same engine stream (engine
    sequencers execute in order, so the stall transfers).

    DMA / collective instructions are processed by the DGE ring / TOPSP, which
    a preceding stream stall does not reliably gate. For those, the inserted
    EventSemaphores absorb ALL original waits and the last one increments a
    per-engine gate semaphore; the ring instruction then carries the single
    gate wait."""
    ctr = 0
    counts: dict[int, int] = {}
    for fn in nc.m.functions:
        for bb in fn.blocks:
            out = []
            changed = False
            for ins in bb.instructions:
                si = ins.sync_info
                if si is None or len(si.on_wait) <= 1:
                    out.append(ins)
                    continue
                changed = True
                waits = list(si.on_wait)
                eng = ins.engine
                if isinstance(ins, _RING_INSTS):
                    h = gate_sems[eng]
                    cnt = counts.get(h.num, 0) + 1
                    counts[h.num] = cnt
                    for j, w in enumerate(waits):
                        ctr += 1
                        ev = mybir.InstEventSemaphore(
                            name=f"gate-ev-{ctr}", engine=eng)
                        upd = []
                        if j == len(waits) - 1:
                            upd = [bass_rust.SyncUpdate(
                                sync_type="semaphore", id=h.num, ant_name=h.name,
                                update_mode="sem-inc", update_value=1,
                                update_reg=None)]
                        ev.sync_info = bass_rust.SyncInfo(on_wait=[w], on_update=upd)
                        out.append(ev)
                    ins.sync_info = bass_rust.SyncInfo(
                        on_wait=[bass_rust.SyncWait(
                            sync_type="semaphore", id=h.num, ant_name=h.name,
                            wait_mode="sem-ge-imm", wait_value=cnt,
                            wait_reg=None)],
                        on_update=list(si.on_update),
                    )
                else:
                    for w in waits[:-1]:
                        ctr += 1
                        ev = mybir.InstEventSemaphore(
                            name=f"gate-ev-{ctr}", engine=eng)
                        ev.sync_info = bass_rust.SyncInfo(on_wait=[w], on_update=[])
                        out.append(ev)
                    ins.sync_info = bass_rust.SyncInfo(
                        on_wait=[waits[-1]], on_update=list(si.on_update)
                    )
                out.append(ins)
            if changed:
                bb.instructions = out


def _build_nc(skip_a2a: bool = False) -> bass.Bass:
    nc = bass.Bass()
    gate_sems = {
        e: nc.alloc_semaphore(f"mw_gate_{i}")
        for i, e in enumerate([
            mybir.EngineType.SP, mybir.EngineType.Pool,
            mybir.EngineType.Activation, mybir.EngineType.PE,
            mybir.EngineType.DVE,
        ])
    }

    # DRAM parameters (bf16 compute inputs prepared host-side)
    xT = nc.declare_dram_parameter("xT", [C, B * N], BF16, isOutput=False)
    wq = nc.declare_dram_parameter("wq", [C, P], BF16, isOutput=False)
    wk = nc.declare_dram_parameter("wk", [C, P], BF16, isOutput=False)
    wv = nc.declare_dram_parameter("wv", [C, P], BF16, isOutput=False)
    bq = nc.declare_dram_parameter("bq", [P, 1], F32, isOutput=False)   # pre-scaled
    bk = nc.declare_dram_parameter("bk", [P, 1], F32, isOutput=False)
    bvr = nc.declare_dram_parameter("bvr", [P, P], F32, isOutput=False)  # replicated
    wp = nc.declare_dram_parameter("wp", [C, C], BF16, isOutput=False)
    bp = nc.declare_dram_parameter("bp", [P, KT], F32, isOutput=False)  # [p, mtile]
    out = nc.declare_dram_parameter("out", [C, 512], F32, isOutput=True)

    from contextlib import ExitStack
    with tile.TileContext(nc) as tc:
        with (
            tc.tile_pool(name="persist", bufs=1) as pp,
            tc.tile_pool(name="work", bufs=3) as wk_pool,
            tc.tile_pool(name="dram", bufs=2, space="DRAM") as dram,
            ExitStack() as phase_ctx,
        ):
            psq = phase_ctx.enter_context(
                tc.tile_pool(name="psumq", bufs=1, space="PSUM"))
            # ---- persistent SBUF loads ----
            xT_sb = pp.tile([P, KT, B * N], BF16, tag="xT")
            xT_r = xT.rearrange("(kt p) n -> p kt n", p=P)
            wq_sb = pp.tile([P, KT, P], BF16, tag="wq")
            nc.sync.dma_start(wq_sb[:], wq.rearrange("(kt p) m -> p kt m", p=P))
            wk_sb = pp.tile([P, KT, P], BF16, tag="wk")
            nc.sync.dma_start(wk_sb[:], wk.rearrange("(kt p) m -> p kt m", p=P))
            wv_sb = pp.tile([P, KT, P], BF16, tag="wv")
            nc.sync.dma_start(wv_sb[:], wv.rearrange("(kt p) m -> p kt m", p=P))
            bq_sb = pp.tile([P, 1], F32, tag="bq")
            nc.sync.dma_start(bq_sb[:], bq[:])
            bk_sb = pp.tile([P, 1], F32, tag="bk")
            nc.sync.dma_start(bk_sb[:], bk[:])
            bvr_sb = pp.tile([P, P], F32, tag="bvr")
            nc.sync.dma_start(bvr_sb[:], bvr[:])
            wp_sb = pp.tile([P, KT, C], BF16, tag="wp")
            nc.sync.dma_start(wp_sb[:], wp.rearrange("(kt p) m -> p kt m", p=P))
            bp_sb = pp.tile([P, KT], F32, tag="bp")
            nc.sync.dma_start(bp_sb[:], bp[:])

            # persistent activations
            qT_sb = pp.tile([P, B, N], BF16, tag="qT")    # rows: hA 0-63, hB 64-127
            kT_sb = pp.tile([P, B, N], BF16, tag="kT")
            # v_ext: [seq128, b, seqtile, head, 128]; col D is the ones
            # column, cols D+1..127 are zero padding so the av matmul loads a
            # full 128-wide stationary operand (HAM counts it as array-busy)
            v_sb = pp.tile([P, B, N // P, HL, P], BF16, tag="v")
            nc.vector.memset(v_sb[:], 0.0)
            nc.vector.memset(
                v_sb.rearrange("p b s h d -> p (b s h) d")[:, :, D: D + 1], 1.0)

            a2a_in = dram.tile([C, 512], BF16)
            a2a_out = dram.tile([C, 512], BF16)

            # ---- phase 1: qkv ----
            # per-ktile xT loads so matmuls start as soon as slice 0 lands
            for kt in range(KT):
                nc.sync.dma_start(xT_sb[:, kt], xT_r[:, kt])
            for b in range(B):
                for st in range(N // P):
                    ps = psq.tile([P, P], F32, tag="qk0", name=f"ps_v_{b}_{st}")
                    for kt in range(KT):
                        nc.tensor.matmul(
                            ps[:], xT_sb[:, kt, b * N + P * st: b * N + P * (st + 1)], wv_sb[:, kt],
                            start=(kt == 0), stop=(kt == KT - 1),
                        )
                    # v + bias -> [128, HL, D] slices (skip ones column)
                    nc.vector.tensor_tensor(
                        v_sb[:, b, st, :, 0:D],
                        ps.rearrange("p (h d) -> p h d", h=HL),
                        bvr_sb.rearrange("p (h d) -> p h d", h=HL),
                        mybir.AluOpType.add,
                    )

            # q then k, kt-outer over 8 concurrent psum chunks (b x chunk)
            for name, w_sb in (("q", wq_sb), ("k", wk_sb)):
                pss = [
                    psq.tile([P, 512], F32, tag=f"qk{j}", name=f"ps_{name}_{j}")
                    for j in range(8)
                ]
                for kt in range(KT):
                    for j in range(8):
                        b, c = j // NQC, j % NQC
                        nc.tensor.matmul(
                            pss[j][:], w_sb[:, kt],
                            xT_sb[:, kt, b * N + 512 * c: b * N + 512 * (c + 1)],
                            start=(kt == 0), stop=(kt == KT - 1),
                        )
                for j in range(8):
                    b, c = j // NQC, j % NQC
                    if name == "q":
                        nc.vector.tensor_scalar(
                            qT_sb[:, b, 512 * c: 512 * (c + 1)], pss[j][:],
                            SCALE, bq_sb[:], mybir.AluOpType.mult, mybir.AluOpType.add,
                        )
                    else:
                        nc.vector.tensor_scalar_add(
                            kT_sb[:, b, 512 * c: 512 * (c + 1)], pss[j][:], bk_sb[:],
                        )
            # close qkv psum pool; open attention pools
            phase_ctx.close()
            psp = phase_ctx.enter_context(
                tc.tile_pool(name="psum", bufs=2, space="PSUM"))
            psp2 = phase_ctx.enter_context(
                tc.tile_pool(name="psum2", bufs=1, space="PSUM"))

            # warm bridge: dependency-free dense matmuls that the
            # scheduler can run while the k copybacks drain, so the PE has no
            # idle window at attention entry (idle re-throttles the HAM clock
            # gate and the first ~70us of attention would run at half rate)
            warm_ps = psp.tile([P, 1024], F32, tag="s", name="warm_ps")
            for r in range(3):
                for c in range(8):
                    nc.tensor.matmul(
                        warm_ps[:, 512 * (c % 2): 512 * (c % 2 + 1)],
                        wp_sb[:, 0, 0:P],
                        xT_sb[:, r, 512 * (c % 4): 512 * (c % 4 + 1)],
                        start=True, stop=True,
                    )
            warm_anchor = wk_pool.tile([1, 8], F32, tag="warm_anchor")
            nc.vector.tensor_copy(warm_anchor[:], warm_ps[0:1, 0:8])

            # ---- phase 2: attention ----
            # One (batch, head) at a time over the full 2048-query row:
            # one LDWEIGHTS feeds 4 matmuls (keeps PE array duty high so the
            # HAM clock gate stays at full rate), split score buffers let ACT
            # exp overlap the next nk-tile's scores.
            for b in range(B):
                for h in range(HL):
                    ps_o = psp2.tile([P, N], F32, tag="o", name=f"ps_o_{b}_{h}")
                    for nk in range(N // P):
                        ps_s = [
                            psp.tile([P, 1024], F32, tag="s",
                                     name=f"ps_s_{b}_{h}_{nk}_{i}")
                            for i in range(2)
                        ]
                        for c in range(4):
                            nc.tensor.matmul(
                                ps_s[c // 2][:, 512 * (c % 2): 512 * (c % 2 + 1)],
                                kT_sb[D * h: D * (h + 1), b, P * nk: P * (nk + 1)],
                                qT_sb[D * h: D * (h + 1), b, 512 * c: 512 * (c + 1)],
                                start=True, stop=True,
                                tile_position=(D * h, 0),
                            )
                        exps = []
                        for i in range(2):
                            e = wk_pool.tile([P, 1024], BF16, tag="exp")
                            exps.append(e)
                            nc.scalar.activation(
                                e[:], ps_s[i][:], mybir.ActivationFunctionType.Exp,
                            )
                        for c in range(4):
                            nc.tensor.matmul(
                                ps_o[:, 512 * c: 512 * (c + 1)],
                                v_sb[:, b, nk, h],
                                exps[c // 2][:, 512 * (c % 2): 512 * (c % 2 + 1)],
                                start=(nk == 0), stop=(nk == N // P - 1),
                            )
                    # normalize rows 0..D-1 by row D; emit to a2a_in
                    nd_sb = wk_pool.tile([D + 1, N], F32, tag="ndsb")
                    nc.vector.tensor_copy(nd_sb[:], ps_o[0: D + 1, :])
                    d_dram = dram.tile([1, N], F32, name=f"dd_{b}_{h}")
                    nc.sync.dma_start(d_dram[:], nd_sb[D: D + 1, :])
                    rsc = wk_pool.tile([D, N // D], F32, tag="rsc")
                    nc.sync.dma_start(
                        rsc[:], d_dram.rearrange("o (p f) -> (o p) f", p=D))
                    rscr = wk_pool.tile([D, N // D], F32, tag="rscr")
                    nc.vector.reciprocal(rscr[:], rsc[:])
                    r_dram = dram.tile([D, N // D], F32, name=f"rd_{b}_{h}")
                    nc.sync.dma_start(r_dram[:], rscr[:])
                    bc_sb = wk_pool.tile([D, N], F32, tag="bcsb")
                    nc.sync.dma_start(
                        bc_sb[:, None, :],
                        r_dram.rearrange("p f -> (p f)")[None, :].partition_broadcast(D))
                    o_sb = wk_pool.tile([D, N], BF16, tag="osb")
                    nc.vector.tensor_tensor(
                        o_sb[:], nd_sb[0:D, :], bc_sb[:], mybir.AluOpType.mult,
                    )
                    nc.sync.dma_start(
                        a2a_in.rearrange("(j r) n -> r j n", r=P)[
                            D * h: D * (h + 1), 4 * b: 4 * b + 4, :,
                        ],
                        o_sb.rearrange("d (c n) -> d c n", n=512),
                    )

            # ---- phase 3: all-to-all ----
            if skip_a2a:
                nc.sync.dma_start(a2a_out[:], a2a_in[:])
            else:
                nc.gpsimd.collective_compute(
                "AllToAll",
                mybir.AluOpType.bypass,
                    replica_groups=[list(range(W))],
                    ins=[a2a_in.opt()],
                    outs=[a2a_out.opt()],
                )

            # ---- phase 4: projection for this core's (b, chunk) ----
            rx_sb = pp.tile([P, KT, 512], BF16, tag="rx")
            nc.sync.dma_start(rx_sb[:], a2a_out.rearrange("(kt p) n -> p kt n", p=P))
            for mt in range(KT):
                ps = psp.tile([P, 512], F32, tag="s", name=f"ps_proj_{mt}")
                for kt in range(KT):
                    nc.tensor.matmul(
                        ps[:], wp_sb[:, kt, P * mt: P * (mt + 1)], rx_sb[:, kt],
                        start=(kt == 0), stop=(kt == KT - 1),
                    )
                o_sb = wk_pool.tile([P, 512], F32, tag="proj")
                nc.vector.tensor_scalar_add(o_sb[:], ps[:], bp_sb[:, mt: mt + 1])
                nc.sync.dma_start(out[P * mt: P * (mt + 1), :], o_sb[:])

    _split_multiwait(nc, gate_sems)
    return nc


_NC_CACHE: bass.Bass | None = None


def _get_nc() -> bass.Bass:
    global _NC_CACHE
    if _NC_CACHE is None:
        _NC_CACHE = _build_nc()
    return _NC_CACHE


def _prep_inputs(x, qkv_w, qkv_b, proj_w, proj_b):
    x = np.asarray(x, dtype=np.float32)
    qkv_w = np.asarray(qkv_w, dtype=np.float32)
    qkv_b = np.asarray(qkv_b, dtype=np.float32)
    proj_w = np.asarray(proj_w, dtype=np.float32)
    proj_b = np.asarray(proj_b, dtype=np.float32)

    # [C, B*N], bf16
    xT = np.ascontiguousarray(
        np.concatenate([x[b].T for b in range(B)], axis=1)
    ).astype(BF)
    wp = np.ascontiguousarray(proj_w).astype(BF)
    bp = np.ascontiguousarray(proj_b.reshape(KT, P).T)  # [p, mtile]

    in_maps = []
    for i in range(W):
        ch0 = P * i  # first channel of this core's head pair
        wq_i = np.ascontiguousarray(qkv_w[:, ch0: ch0 + P]).astype(BF)
        wk_i = np.ascontiguousarray(qkv_w[:, C + ch0: C + ch0 + P]).astype(BF)
        wv_i = np.ascontiguousarray(qkv_w[:, 2 * C + ch0: 2 * C + ch0 + P]).astype(BF)
        bq_i = np.ascontiguousarray(
            (qkv_b[ch0: ch0 + P] * SCALE).reshape(P, 1)
        )
        bk_i = np.ascontiguousarray(qkv_b[C + ch0: C + ch0 + P].reshape(P, 1))
        bv_i = np.ascontiguousarray(
            np.broadcast_to(qkv_b[2 * C + ch0: 2 * C + ch0 + P], (P, P))
        )
        in_maps.append({
            "xT": xT, "wq": wq_i, "wk": wk_i, "wv": wv_i,
            "bq": bq_i, "bk": bk_i, "bvr": bv_i,
            "wp": wp, "bp": bp,
        })
    return in_maps


def kernel(x, qkv_w, qkv_b, proj_w, proj_b, _trace=False, _trace_kwargs=None):
    nc = _get_nc()
    in_maps = _prep_inputs(x, qkv_w, qkv_b, proj_w, proj_b)
    res = run_bass_kernel_spmd(
        nc, in_maps, list(range(W)), trace=_trace, **(_trace_kwargs or {})
    )
    out = np.empty((B, N, C), dtype=np.float32)
    for i in range(W):
        b, g = i // 4, i % 4
        out[b, 512 * g: 512 * (g + 1), :] = res.results[i]["out"].T
    kernel._last_result = res
    return out


# revision 24
# speedup vs baseline: 1.1411x; 1.1411x over previous
"""Multi-head self-attention (B=2, N=2048, C=1024, H=16) on 8 trn2 NeuronCores.

Sharding: core i computes heads {2i, 2i+1} for both batches (head-parallel
attention), then one 8-way AllToAll redistributes attention outputs so core i
holds the full channel dim for output rows (b = i//4, seq chunk 512*(i%4)).
Each core then computes its own 512-row slice of the output projection.
Host work is layout only: dtype cast, transpose, slice, concatenate.
"""

import numpy as np
import ml_dtypes
import bass_rust

import concourse.bass as bass
import concourse.mybir as mybir
import concourse.tile as tile
from concourse.bass_utils import run_bass_kernel_spmd

B, N, C = 2, 2048, 1024
H = 16
D = C // H           # 64
W = 8                # cores
HL = 2               # heads per core
P = 128
KT = C // P          # 8 k-tiles over channels
NQC = N // 512       # 4 query chunks of 512 per batch
SCALE = float(D) ** -0.5

F32 = mybir.dt.float32
BF16 = mybir.dt.bfloat16
BF = ml_dtypes.bfloat16


_RING_INSTS = (
    mybir.InstDMACopy, mybir.InstDMA, mybir.InstTensorLoad, mybir.InstTensorSave,
    mybir.InstDmaTransposeAnt, mybir.InstDMAGatherAnt, mybir.InstDMAScatterAddAnt,
    mybir.InstCollectiveCompute,
)


def _split_multiwait(nc: bass.Bass, gate_sems: dict) -> None:
    """This toolchain's walrus codegen accepts at most ONE sync wait per
    instruction, but the Tile scheduler attaches several.

    Compute/CTRL instructions: move all but the last wait onto EventSemaphore
    instructions inserted just before them on the same engine stream (engine
    sequencers execute in order, so the stall transfers).

    DMA / collective instructions are processed by the DGE ring / TOPSP, which
    a preceding stream stall does not reliably gate. For those, the inserted
    EventSemaphores absorb ALL original waits and the last one increments a
    per-engine gate semaphore; the ring instruction then carries the single
    gate wait."""
    ctr = 0
    counts: dict[int, int] = {}
    for fn in nc.m.functions:
        for bb in fn.blocks:
            out = []
            changed = False
            for ins in bb.instructions:
                si = ins.sync_info
                if si is None or len(si.on_wait) <= 1:
                    out.append(ins)
                    continue
                changed = True
                waits = list(si.on_wait)
                eng = ins.engine
                if isinstance(ins, _RING_INSTS):
                    h = gate_sems[eng]
                    cnt = counts.get(h.num, 0) + 1
                    counts[h.num] = cnt
                    for j, w in enumerate(waits):
                        ctr += 1
                        ev = mybir.InstEventSemaphore(
                            name=f"gate-ev-{ctr}", engine=eng)
                        upd = []
                        if j == len(waits) - 1:
                            upd = [bass_rust.SyncUpdate(
                                sync_type="semaphore", id=h.num, ant_name=h.name,
                                update_mode="sem-inc", update_value=1,
                                update_reg=None)]
                        ev.sync_info = bass_rust.SyncInfo(on_wait=[w], on_update=upd)
                        out.append(ev)
                    ins.sync_info = bass_rust.SyncInfo(
                        on_wait=[bass_rust.SyncWait(
                            sync_type="semaphore", id=h.num, ant_name=h.name,
                            wait_mode="sem-ge-imm", wait_value=cnt,
                            wait_reg=None)],
                        on_update=list(si.on_update),
                    )
                else:
                    for w in waits[:-1]:
                        ctr += 1
                        ev = mybir.InstEventSemaphore(
                            name=f"gate-ev-{ctr}", engine=eng)
                        ev.sync_info = bass_rust.SyncInfo(on_wait=[w], on_update=[])
                        out.append(ev)
                    ins.sync_info = bass_rust.SyncInfo(
                        on_wait=[waits[-1]], on_update=list(si.on_update)
                    )
                out.append(ins)
            if changed:
                bb.instructions = out


def _build_nc(skip_a2a: bool = False) -> bass.Bass:
    nc = bass.Bass()
    gate_sems = {
        e: nc.alloc_semaphore(f"mw_gate_{i}")
        for i, e in enumerate([
            mybir.EngineType.SP, mybir.EngineType.Pool,
            mybir.EngineType.Activation, mybir.EngineType.PE,
            mybir.EngineType.DVE,
        ])
    }

    # DRAM parameters (bf16 compute inputs prepared host-side)
    xT = nc.declare_dram_parameter("xT", [C, B * N], BF16, isOutput=False)
    wq = nc.declare_dram_parameter("wq", [C, P], BF16, isOutput=False)
    wk = nc.declare_dram_parameter("wk", [C, P], BF16, isOutput=False)
    wv = nc.declare_dram_parameter("wv", [C, P], BF16, isOutput=False)
    bq = nc.declare_dram_parameter("bq", [P, 1], F32, isOutput=False)   # pre-scaled
    bk = nc.declare_dram_parameter("bk", [P, 1], F32, isOutput=False)
    bvr = nc.declare_dram_parameter("bvr", [P, P], F32, isOutput=False)  # replicated
    wp = nc.declare_dram_parameter("wp", [C, C], BF16, isOutput=False)
    bp = nc.declare_dram_parameter("bp", [P, KT], F32, isOutput=False)  # [p, mtile]
    out = nc.declare_dram_parameter("out", [C, 512], F32, isOutput=True)

    from contextlib import ExitStack
    with tile.TileContext(nc) as tc:
        with (
            tc.tile_pool(name="persist", bufs=1) as pp,
            tc.tile_pool(name="work", bufs=3) as wk_pool,
            tc.tile_pool(name="dram", bufs=2, space="DRAM") as dram,
            ExitStack() as phase_ctx,
        ):
            psq = phase_ctx.enter_context(
                tc.tile_pool(name="psumq", bufs=1, space="PSUM"))
            # ---- persistent SBUF loads ----
            xT_sb = pp.tile([P, KT, B * N], BF16, tag="xT")
            xT_r = xT.rearrange("(kt p) n -> p kt n", p=P)
            wq_sb = pp.tile([P, KT, P], BF16, tag="wq")
            nc.sync.dma_start(wq_sb[:], wq.rearrange("(kt p) m -> p kt m", p=P))
            wk_sb = pp.tile([P, KT, P], BF16, tag="wk")
            nc.sync.dma_start(wk_sb[:], wk.rearrange("(kt p) m -> p kt m", p=P))
            wv_sb = pp.tile([P, KT, P], BF16, tag="wv")
            nc.sync.dma_start(wv_sb[:], wv.rearrange("(kt p) m -> p kt m", p=P))
            bq_sb = pp.tile([P, 1], F32, tag="bq")
            nc.sync.dma_start(bq_sb[:], bq[:])
            bk_sb = pp.tile([P, 1], F32, tag="bk")
            nc.sync.dma_start(bk_sb[:], bk[:])
            bvr_sb = pp.tile([P, P], F32, tag="bvr")
            nc.sync.dma_start(bvr_sb[:], bvr[:])
            wp_sb = pp.tile([P, KT, C], BF16, tag="wp")
            nc.sync.dma_start(wp_sb[:], wp.rearrange("(kt p) m -> p kt m", p=P))
            bp_sb = pp.tile([P, KT], F32, tag="bp")
            nc.sync.dma_start(bp_sb[:], bp[:])

            # persistent activations
            qT_sb = pp.tile([P, B, N], BF16, tag="qT")    # rows: hA 0-63, hB 64-127
            kT_sb = pp.tile([P, B, N], BF16, tag="kT")
            # v_ext: [seq128, b, seqtile, head, 128]; col D is the ones
            # column, cols D+1..127 are zero padding so the av matmul loads a
            # full 128-wide stationary operand (HAM counts it as array-busy)
            v_sb = pp.tile([P, B, N // P, HL, P], BF16, tag="v")
            nc.vector.memset(v_sb[:], 0.0)
            nc.vector.memset(
                v_sb.rearrange("p b s h d -> p (b s h) d")[:, :, D: D + 1], 1.0)

            a2a_in = dram.tile([C, 512], BF16)
            a2a_out = dram.tile([C, 512], BF16)

            # ---- phase 1: qkv ----
            # per-ktile xT loads so matmuls start as soon as slice 0 lands
            for kt in range(KT):
                nc.sync.dma_start(xT_sb[:, kt], xT_r[:, kt])
            for b in range(B):
                for st in range(N // P):
                    ps = psq.tile([P, P], F32, tag="qk0", name=f"ps_v_{b}_{st}")
                    for kt in range(KT):
                        nc.tensor.matmul(
                            ps[:], xT_sb[:, kt, b * N + P * st: b * N + P * (st + 1)], wv_sb[:, kt],
                            start=(kt == 0), stop=(kt == KT - 1),
                        )
                    # v + bias -> [128, HL, D] slices (skip ones column)
                    nc.vector.tensor_tensor(
                        v_sb[:, b, st, :, 0:D],
                        ps.rearrange("p (h d) -> p h d", h=HL),
                        bvr_sb.rearrange("p (h d) -> p h d", h=HL),
                        mybir.AluOpType.add,
                    )

            # q then k, kt-outer over 8 concurrent psum chunks (b x chunk)
            for name, w_sb in (("q", wq_sb), ("k", wk_sb)):
                pss = [
                    psq.tile([P, 512], F32, tag=f"qk{j}", name=f"ps_{name}_{j}")
                    for j in range(8)
                ]
                for kt in range(KT):
                    for j in range(8):
                        b, c = j // NQC, j % NQC
                        nc.tensor.matmul(
                            pss[j][:], w_sb[:, kt],
                            xT_sb[:, kt, b * N + 512 * c: b * N + 512 * (c + 1)],
                            start=(kt == 0), stop=(kt == KT - 1),
                        )
                for j in range(8):
                    b, c = j // NQC, j % NQC
                    if name == "q":
                        nc.vector.tensor_scalar(
                            qT_sb[:, b, 512 * c: 512 * (c + 1)], pss[j][:],
                            SCALE, bq_sb[:], mybir.AluOpType.mult, mybir.AluOpType.add,
                        )
                    else:
                        nc.vector.tensor_scalar_add(
                            kT_sb[:, b, 512 * c: 512 * (c + 1)], pss[j][:], bk_sb[:],
                        )
            # close qkv psum pool; open attention pools
            phase_ctx.close()
            psp = phase_ctx.enter_context(
                tc.tile_pool(name="psum", bufs=2, space="PSUM"))
            psp2 = phase_ctx.enter_context(
                tc.tile_pool(name="psum2", bufs=1, space="PSUM"))

            # ---- phase 2: attention ----
            # One (batch, head) at a time over the full 2048-query row:
            # one LDWEIGHTS feeds 4 matmuls (keeps PE array duty high so the
            # HAM clock gate stays at full rate), split score buffers let ACT
            # exp overlap the next nk-tile's scores.
            for b in range(B):
                for h in range(HL):
                    ps_o = psp2.tile([P, N], F32, tag="o", name=f"ps_o_{b}_{h}")
                    for nk in range(N // P):
                        ps_s = [
                            psp.tile([P, 1024], F32, tag="s",
                                     name=f"ps_s_{b}_{h}_{nk}_{i}")
                            for i in range(2)
                        ]
                        for c in range(4):
                            nc.tensor.matmul(
                                ps_s[c // 2][:, 512 * (c % 2): 512 * (c % 2 + 1)],
                                kT_sb[D * h: D * (h + 1), b, P * nk: P * (nk + 1)],
                                qT_sb[D * h: D * (h + 1), b, 512 * c: 512 * (c + 1)],
                                start=True, stop=True,
                                tile_position=(D * h, 0),
                            )
                        exps = []
                        for i in range(2):
                            e = wk_pool.tile([P, 1024], BF16, tag="exp")
                            exps.append(e)
                            nc.scalar.activation(
                                e[:], ps_s[i][:], mybir.ActivationFunctionType.Exp,
                            )
                        for c in range(4):
                            nc.tensor.matmul(
                                ps_o[:, 512 * c: 512 * (c + 1)],
                                v_sb[:, b, nk, h],
                                exps[c // 2][:, 512 * (c % 2): 512 * (c % 2 + 1)],
                                start=(nk == 0), stop=(nk == N // P - 1),
                            )
                    # normalize rows 0..D-1 by row D; emit to a2a_in
                    nd_sb = wk_pool.tile([D + 1, N], F32, tag="ndsb")
                    nc.vector.tensor_copy(nd_sb[:], ps_o[0: D + 1, :])
                    d_dram = dram.tile([1, N], F32, name=f"dd_{b}_{h}")
                    nc.sync.dma_start(d_dram[:], nd_sb[D: D + 1, :])
                    rsc = wk_pool.tile([D, N // D], F32, tag="rsc")
                    nc.sync.dma_start(
                        rsc[:], d_dram.rearrange("o (p f) -> (o p) f", p=D))
                    rscr = wk_pool.tile([D, N // D], F32, tag="rscr")
                    nc.vector.reciprocal(rscr[:], rsc[:])
                    r_dram = dram.tile([D, N // D], F32, name=f"rd_{b}_{h}")
                    nc.sync.dma_start(r_dram[:], rscr[:])
                    bc_sb = wk_pool.tile([D, N], F32, tag="bcsb")
                    nc.sync.dma_start(
                        bc_sb[:, None, :],
                        r_dram.rearrange("p f -> (p f)")[None, :].partition_broadcast(D))
                    o_sb = wk_pool.tile([D, N], BF16, tag="osb")
                    nc.vector.tensor_tensor(
                        o_sb[:], nd_sb[0:D, :], bc_sb[:], mybir.AluOpType.mult,
                    )
                    nc.sync.dma_start(
                        a2a_in.rearrange("(j r) n -> r j n", r=P)[
                            D * h: D * (h + 1), 4 * b: 4 * b + 4, :,
                        ],
                        o_sb.rearrange("d (c n) -> d c n", n=512),
                    )

            # ---- phase 3: all-to-all ----
            if skip_a2a:
                nc.sync.dma_start(a2a_out[:], a2a_in[:])
            else:
                nc.gpsimd.collective_compute(
                "AllToAll",
                mybir.AluOpType.bypass,
                    replica_groups=[list(range(W))],
                    ins=[a2a_in.opt()],
                    outs=[a2a_out.opt()],
                )

            # ---- phase 4: projection for this core's (b, chunk) ----
            rx_sb = pp.tile([P, KT, 512], BF16, tag="rx")
            nc.sync.dma_start(rx_sb[:], a2a_out.rearrange("(kt p) n -> p kt n", p=P))
            for mt in range(KT):
                ps = psp.tile([P, 512], F32, tag="s", name=f"ps_proj_{mt}")
                for kt in range(KT):
                    nc.tensor.matmul(
                        ps[:], wp_sb[:, kt, P * mt: P * (mt + 1)], rx_sb[:, kt],
                        start=(kt == 0), stop=(kt == KT - 1),
                    )
                o_sb = wk_pool.tile([P, 512], F32, tag="proj")
                nc.vector.tensor_scalar_add(o_sb[:], ps[:], bp_sb[:, mt: mt + 1])
                nc.sync.dma_start(out[P * mt: P * (mt + 1), :], o_sb[:])

    _split_multiwait(nc, gate_sems)
    return nc


_NC_CACHE: bass.Bass | None = None


def _get_nc() -> bass.Bass:
    global _NC_CACHE
    if _NC_CACHE is None:
        _NC_CACHE = _build_nc()
    return _NC_CACHE


def _prep_inputs(x, qkv_w, qkv_b, proj_w, proj_b):
    x = np.asarray(x, dtype=np.float32)
    qkv_w = np.asarray(qkv_w, dtype=np.float32)
    qkv_b = np.asarray(qkv_b, dtype=np.float32)
    proj_w = np.asarray(proj_w, dtype=np.float32)
    proj_b = np.asarray(proj_b, dtype=np.float32)

    # [C, B*N], bf16
    xT = np.ascontiguousarray(
        np.concatenate([x[b].T for b in range(B)], axis=1)
    ).astype(BF)
    wp = np.ascontiguousarray(proj_w).astype(BF)
    bp = np.ascontiguousarray(proj_b.reshape(KT, P).T)  # [p, mtile]

    in_maps = []
    for i in range(W):
        ch0 = P * i  # first channel of this core's head pair
        wq_i = np.ascontiguousarray(qkv_w[:, ch0: ch0 + P]).astype(BF)
        wk_i = np.ascontiguousarray(qkv_w[:, C + ch0: C + ch0 + P]).astype(BF)
        wv_i = np.ascontiguousarray(qkv_w[:, 2 * C + ch0: 2 * C + ch0 + P]).astype(BF)
        bq_i = np.ascontiguousarray(
            (qkv_b[ch0: ch0 + P] * SCALE).reshape(P, 1)
        )
        bk_i = np.ascontiguousarray(qkv_b[C + ch0: C + ch0 + P].reshape(P, 1))
        bv_i = np.ascontiguousarray(
            np.broadcast_to(qkv_b[2 * C + ch0: 2 * C + ch0 + P], (P, P))
        )
        in_maps.append({
            "xT": xT, "wq": wq_i, "wk": wk_i, "wv": wv_i,
            "bq": bq_i, "bk": bk_i, "bvr": bv_i,
            "wp": wp, "bp": bp,
        })
    return in_maps


def kernel(x, qkv_w, qkv_b, proj_w, proj_b, _trace=False, _trace_kwargs=None):
    nc = _get_nc()
    in_maps = _prep_inputs(x, qkv_w, qkv_b, proj_w, proj_b)
    res = run_bass_kernel_spmd(
        nc, in_maps, list(range(W)), trace=_trace, **(_trace_kwargs or {})
    )
    out = np.empty((B, N, C), dtype=np.float32)
    for i in range(W):
        b, g = i // 4, i % 4
        out[b, 512 * g: 512 * (g + 1), :] = res.results[i]["out"].T
    kernel._last_result = res
    return out
